# revision 1
# baseline (speedup 1.0000x reference)
"""Debayer 3x3 kernel for Trainium2 (Bass/Tile), batch-sharded over 8 NeuronCores.

Reference semantics: 1->5 channel 3x3 conv (identity, plus-4, diag-4,
horiz-2, vert-2) over an edge-padded Bayer frame, then per-2x2-parity
channel select into RGB.

Per-pixel, with q = x/4:
  SQ = q[left]+q[right]  (= H/4)     VQ = q[up]+q[down]  (= V/4)
  c0 = x = 4q   c1 = SQ+VQ   c2 = SQ[up]+SQ[down]   c3 = 2*SQ   c4 = 2*VQ
RGB parity table (row parity, col parity):
  R: (e,e)=c0 (e,o)=c3 (o,e)=c4 (o,o)=c2
  G: (e,e)=c1 (e,o)=c0 (o,e)=c0 (o,o)=c1
  B: (e,e)=c2 (e,o)=c4 (o,e)=c3 (o,o)=c0

Device layout: the host pre-tiles each padded 1090x1922 image into
128 partitions x 4 col-slices x (36 rows x 122 cols) patches:
  partition p = 32*q + b  (col-quarter q in 0..3, row-band b in 0..31)
  band b   -> image rows [34b, 34b+34)        (patch has +-1 halo rows)
  slice s  -> image cols [480q+120s, +120)    (patch has +-1 halo cols)
All stencil shifts are then free-dim AP offsets; parity classes are
stride-2 APs. 34 and 120 are even so parity phase is uniform across
partitions/slices.
"""

import numpy as np

H, W = 1088, 1920
NB = 32          # row bands per column-quarter
BH = 34          # output rows per band
NQ = 4           # column quarters
NS = 4           # col slices per patch
SW = 120         # output cols per slice
PR, PC = BH + 2, SW + 2   # patch rows/cols (with halo)

_NC_CACHE = {}
LAST_RESULTS = None


def _build(reps=1, *, no_compute=False, no_act=False, out_engine="sync",
           in_bufs=2, mid_bufs=2, out_bufs=2, vq_bufs=None,
           gp_adds=False, gp_scale=False):
    """Build the Bass module. reps>1 repeats the whole pipeline (bench only:
    amortizes per-dispatch overhead out of wall-clock measurements)."""
    key = (reps, no_compute, no_act, out_engine, in_bufs, mid_bufs, out_bufs,
           vq_bufs, gp_adds, gp_scale)
    if key in _NC_CACHE:
        return _NC_CACHE[key]
    import concourse.bacc as bacc
    import concourse.mybir as mybir
    import concourse.tile as tile
    from concourse._compat import get_trn_type

    f32 = mybir.dt.float32
    nc = bacc.Bacc(get_trn_type() or "TRN2", target_bir_lowering=False, debug=False)
    xin = nc.dram_tensor("xprep", [128, NS, PR, PC], f32, kind="ExternalInput")
    yout = nc.dram_tensor("yout", [3, 128, NS, BH, SW], f32, kind="ExternalOutput")
    # bench-only: earlier reps dump to internal scratch so no two reps write
    # the same DRAM (WAW races hang the exec unit)
    ydumps = [
        nc.dram_tensor(f"ydump{r}", [3, 128, NS, BH, SW], f32, kind="Internal")
        for r in range(reps - 1)
    ]

    # out-row/out-col parity slices (within [BH, SW] output tiles)
    E_, O_ = slice(0, BH, 2), slice(1, BH, 2)
    e_, o_ = slice(0, SW, 2), slice(1, SW, 2)
    # patch-row slice for out rows of given parity (out row i -> patch row i+1)
    pE, pO = slice(1, PR - 1, 2), slice(2, PR, 2)
    # patch-col slice for out cols of given parity (out col j -> patch col j+1)
    ce, co = slice(1, PC - 1, 2), slice(2, PC, 2)
    # SQ rows for diag channel: out row i needs patch rows i and i+2
    dE0, dE1 = slice(0, PR - 2, 2), slice(2, PR, 2)      # even out rows
    dO0, dO1 = slice(1, PR - 1, 2), slice(3, PR, 2)      # odd out rows

    with tile.TileContext(nc) as tc:
        with tc.tile_pool(name="pin", bufs=in_bufs) as pin, \
             tc.tile_pool(name="pmid", bufs=mid_bufs) as pmid, \
             tc.tile_pool(name="pout", bufs=out_bufs) as pout:

            dma_out = nc.scalar if out_engine == "scalar" else nc.sync

            def load(j):
                t = pin.tile([128, PR, PC], f32, tag="inp", name=f"inp{j}")
                nc.sync.dma_start(out=t[:], in_=xin[:, j % NS])
                return t

            cur = load(0)
            for j in range(NS * reps):
                k = j % NS
                r = j // NS
                ytgt = yout if r == reps - 1 else ydumps[r]
                nxt = load(j + 1) if j + 1 < NS * reps else None
                Q = cur
                R = pout.tile([128, BH, SW], f32, tag="r", name=f"r{k}")
                G = pout.tile([128, BH, SW], f32, tag="g", name=f"g{k}")
                B = pout.tile([128, BH, SW], f32, tag="b", name=f"b{k}")
                if no_compute:
                    # bench-only: DMA skeleton (touch input once so it's live)
                    nc.vector.tensor_copy(R[:, 0:1, 0:SW], Q[:, 0:1, 0:SW])
                    for ch, t in ((0, R), (1, G), (2, B)):
                        dma_out.dma_start(out=ytgt[ch, :, k], in_=t[:])
                    cur = nxt
                    continue
                # prescale in place: Q = x/4
                scale_eng = nc.gpsimd if gp_scale else nc.vector
                scale_eng.tensor_scalar_mul(Q[:], Q[:], 0.25)
                # SQ[p, r, j] = H/4 at patch row r, out col j
                SQ = pmid.tile([128, PR, SW], f32, tag="sq", name=f"sq{k}")
                nc.vector.tensor_add(SQ[:], Q[:, :, 0:SW], Q[:, :, 2:PC])
                # VQ[p, i, j] = V/4 at out row i, out col j
                VQ = pmid.tile([128, BH, SW], f32, tag="vq", name=f"vq{k}",
                               bufs=vq_bufs)
                nc.vector.tensor_add(VQ[:], Q[:, 0:PR - 2, 1:PC - 1], Q[:, 2:PR, 1:PC - 1])

                if no_act:
                    def act_mul(out, in_, s):
                        nc.vector.tensor_scalar_mul(out, in_, s)
                else:
                    act_mul = nc.scalar.mul
                padd = nc.gpsimd if gp_adds else nc.vector
                # ---- R ----
                padd.tensor_add(R[:, O_, o_], SQ[:, dO0, o_], SQ[:, dO1, o_])       # c2
                act_mul(R[:, E_, e_], Q[:, pE, ce], 4.0)                            # c0
                act_mul(R[:, E_, o_], SQ[:, pE, o_], 2.0)                           # c3
                act_mul(R[:, O_, e_], VQ[:, O_, e_], 2.0)                           # c4
                dma_out.dma_start(out=ytgt[0, :, k], in_=R[:])
                # ---- G ----
                padd.tensor_add(G[:, E_, e_], SQ[:, pE, e_], VQ[:, E_, e_])         # c1
                padd.tensor_add(G[:, O_, o_], SQ[:, pO, o_], VQ[:, O_, o_])         # c1
                act_mul(G[:, E_, o_], Q[:, pE, co], 4.0)                            # c0
                act_mul(G[:, O_, e_], Q[:, pO, ce], 4.0)                            # c0
                dma_out.dma_start(out=ytgt[1, :, k], in_=G[:])
                # ---- B ----
                padd.tensor_add(B[:, E_, e_], SQ[:, dE0, e_], SQ[:, dE1, e_])       # c2
                act_mul(B[:, E_, o_], VQ[:, E_, o_], 2.0)                           # c4
                act_mul(B[:, O_, e_], SQ[:, pO, e_], 2.0)                           # c3
                act_mul(B[:, O_, o_], Q[:, pO, co], 4.0)                            # c0
                dma_out.dma_start(out=ytgt[2, :, k], in_=B[:])

                cur = nxt

    nc.compile()
    _NC_CACHE[key] = nc
    return nc


def _prep_inputs(x):
    """(B,1,1088,1920) -> (B,128,NS,PR,PC) patch layout (edge padded)."""
    Bn = x.shape[0]
    xpad = np.pad(x[:, 0], ((0, 0), (1, 1), (1, 1)), mode="edge")  # (B,1090,1922)
    xprep = np.empty((Bn, 128, NS, PR, PC), np.float32)
    st = xpad.strides
    for q in range(NQ):
        for s in range(NS):
            c0 = 480 * q + SW * s
            block = xpad[:, :, c0:c0 + PC]
            v = np.lib.stride_tricks.as_strided(
                block, shape=(Bn, NB, PR, PC),
                strides=(st[0], BH * st[1], st[1], st[2]))
            xprep[:, q * NB:(q + 1) * NB, s] = v
    return xprep


def _assemble(y):
    """(3,128,NS,BH,SW) -> (3,1088,1920)."""
    out = np.empty((3, H, W), np.float32)
    for q in range(NQ):
        rows = y[:, q * NB:(q + 1) * NB]          # (3,NB,NS,BH,SW)
        for s in range(NS):
            c0 = 480 * q + SW * s
            out[:, :, c0:c0 + SW] = rows[:, :, s].reshape(3, H, SW)
    return out


def kernel(x, kernels=None, index=None, **_unused):
    global LAST_RESULTS
    x = np.ascontiguousarray(np.asarray(x), dtype=np.float32)
    Bn = x.shape[0]
    xprep = _prep_inputs(x)
    nc = _build(in_bufs=3, vq_bufs=1)
    from concourse.bass_utils import run_bass_kernel_spmd
    in_maps = [{"xprep": xprep[i]} for i in range(Bn)]
    res = run_bass_kernel_spmd(nc, in_maps, core_ids=list(range(Bn)))
    LAST_RESULTS = res
    out = np.empty((Bn, 3, H, W), np.float32)
    for i in range(Bn):
        out[i] = _assemble(res.results[i]["yout"])
    return out



# revision 2
# speedup vs baseline: 1.4134x; 1.4134x over previous
"""Debayer 3x3 kernel for Trainium2 (Bass/Tile), batch-sharded over 8 NeuronCores.

Reference semantics: 1->5 channel 3x3 conv (identity, plus-4, diag-4,
horiz-2, vert-2) over an edge-padded Bayer frame, then per-2x2-parity
channel select into RGB.

v2 (memory-optimized): all device I/O in fp16 (tolerance is 2e-2; fp16
adds <1e-3), and the identity channel (1 of every 3 output values equals
the input pixel exactly) is pasted on the host from the original f32
input. The device computes only the 8 non-trivial quarter-resolution
planes per tile, packed contiguously for one large DMA per slice:
  P0 c1_ee->G  P1 c1_oo->G  P2 c2_ee->B  P3 c2_oo->R
  P4 c3_eo->R  P5 c3_oe->B  P6 c4_eo->B  P7 c4_oe->R
Per-pixel, with q = x/4:
  SQ[r,c] = q[r,c]+q[r,c+2]   (horiz pair, centered at out col c)
  VQ[r,c] = q[r,c]+q[r+2,c]   (vert pair, centered at out row r)
  c1 = SQ+VQ   c2 = SQ[up]+SQ[down]   c3 = 2*SQ   c4 = 2*VQ

Device traffic per core: in 128*4*36*122*2B = 4.5 MB, out
128*4*8*17*60*2B = 4.2 MB (vs 34 MB for the f32 3-channel baseline).
Full-res pair sums run on DVE in 2x packed-fp16 mode (step-1, 4B-aligned
APs); the stride-2 parity combines go to gpsimd (adds) and the scalar
activation engine (scaled copies), keeping DVE under the DMA roofline.

Device layout: the host pre-tiles each padded 1090x1922 fp16 image into
128 partitions x 4 col-slices x (36 rows x 122 cols) patches:
  partition p = 32*q + b  (col-quarter q in 0..3, row-band b in 0..31)
  band b   -> image rows [34b, 34b+34)        (patch has +-1 halo rows)
  slice s  -> image cols [480q+120s, +120)    (patch has +-1 halo cols)
34 and 120 are even so parity phase is uniform across partitions/slices.
"""

import numpy as np

H, W = 1088, 1920
NB = 32          # row bands per column-quarter
BH = 34          # output rows per band
NQ = 4           # column quarters
NS = 4           # col slices per patch
SW = 120         # output cols per slice
PR, PC = BH + 2, SW + 2   # patch rows/cols (with halo)
QR, QC = 17, 60           # quarter-res plane dims per tile

# (plane, channel, row parity, col parity) for host-side assembly
PLANES = [
    (0, 1, 0, 0),  # c1_ee -> G
    (1, 1, 1, 1),  # c1_oo -> G
    (2, 2, 0, 0),  # c2_ee -> B
    (3, 0, 1, 1),  # c2_oo -> R
    (4, 0, 0, 1),  # c3_eo -> R
    (5, 2, 1, 0),  # c3_oe -> B
    (6, 2, 0, 1),  # c4_eo -> B
    (7, 0, 1, 0),  # c4_oe -> R
]

_NC_CACHE = {}
LAST_RESULTS = None


def _build(reps=1, *, c12="gpsimd", sc="scalar", in_bufs=3, mid_bufs=2,
           out_bufs=2, **_ignored):
    """Build the Bass module. reps>1 repeats the whole pipeline (bench only:
    amortizes per-dispatch overhead out of wall-clock measurements)."""
    key = (reps, c12, sc, in_bufs, mid_bufs, out_bufs)
    if key in _NC_CACHE:
        return _NC_CACHE[key]
    import concourse.bacc as bacc
    import concourse.mybir as mybir
    import concourse.tile as tile
    from concourse._compat import get_trn_type

    f16 = mybir.dt.float16
    nc = bacc.Bacc(get_trn_type() or "TRN2", target_bir_lowering=False, debug=False)
    xin = nc.dram_tensor("xprep", [128, NS, PR, PC], f16, kind="ExternalInput")
    yout = nc.dram_tensor("yout", [128, NS, 8, QR, QC], f16, kind="ExternalOutput")
    # bench-only: earlier reps dump to internal scratch so no two reps write
    # the same DRAM (WAW races hang the exec unit)
    ydumps = [
        nc.dram_tensor(f"ydump{r}", [128, NS, 8, QR, QC], f16, kind="Internal")
        for r in range(reps - 1)
    ]

    with tile.TileContext(nc) as tc:
        with tc.tile_pool(name="pin", bufs=in_bufs) as pin, \
             tc.tile_pool(name="pmid", bufs=mid_bufs) as pmid, \
             tc.tile_pool(name="pout", bufs=out_bufs) as pout:

            eng12 = {"gpsimd": nc.gpsimd, "vector": nc.vector}[c12]
            if sc == "scalar":
                def smul(out, in_, s):
                    nc.scalar.mul(out, in_, s)
            else:
                eng_s = {"gpsimd": nc.gpsimd, "vector": nc.vector}[sc]
                def smul(out, in_, s):
                    eng_s.tensor_scalar_mul(out, in_, s)

            def load(j):
                t = pin.tile([128, PR, PC], f16, tag="inp", name=f"inp{j}")
                nc.sync.dma_start(out=t[:], in_=xin[:, j % NS])
                return t

            cur = load(0)
            for j in range(NS * reps):
                k = j % NS
                r = j // NS
                ytgt = yout if r == reps - 1 else ydumps[r]
                nxt = load(j + 1) if j + 1 < NS * reps else None
                Q = cur
                # prescale in place: Q = x/4 (TS, 4x packed mode)
                nc.vector.tensor_scalar_mul(Q[:], Q[:], 0.25)
                # SQ[p, r, c] = H/4 centered at out col c (TT, 2x mode)
                SQ = pmid.tile([128, PR, SW], f16, tag="sq", name=f"sq{k}")
                nc.vector.tensor_add(SQ[:], Q[:, :, 0:SW], Q[:, :, 2:PC])
                # VQ[p, r, c] = V/4 centered at out row r, full patch width
                VQ = pmid.tile([128, PR - 2, PC], f16, tag="vq", name=f"vq{k}")
                nc.vector.tensor_add(VQ[:], Q[:, 0:PR - 2, :], Q[:, 2:PR, :])

                Y = pout.tile([128, 8, QR, QC], f16, tag="y", name=f"y{k}")
                # c1 = SQ+VQ, c2 = SQ up+down (stride-2 parity reads)
                eng12.tensor_add(Y[:, 0], SQ[:, 1:35:2, 0:120:2], VQ[:, 0:34:2, 1:121:2])
                eng12.tensor_add(Y[:, 1], SQ[:, 2:36:2, 1:120:2], VQ[:, 1:34:2, 2:122:2])
                eng12.tensor_add(Y[:, 2], SQ[:, 0:34:2, 0:120:2], SQ[:, 2:36:2, 0:120:2])
                eng12.tensor_add(Y[:, 3], SQ[:, 1:35:2, 1:120:2], SQ[:, 3:36:2, 1:120:2])
                # c3 = 2*SQ, c4 = 2*VQ (scaled copies on the act engine)
                smul(Y[:, 4], SQ[:, 1:35:2, 1:120:2], 2.0)
                smul(Y[:, 5], SQ[:, 2:36:2, 0:120:2], 2.0)
                smul(Y[:, 6], VQ[:, 0:34:2, 2:122:2], 2.0)
                smul(Y[:, 7], VQ[:, 1:34:2, 1:121:2], 2.0)
                nc.sync.dma_start(out=ytgt[:, k], in_=Y[:])

                cur = nxt

    nc.compile()
    _NC_CACHE[key] = nc
    return nc


def _prep_inputs(x):
    """(B,1,1088,1920) f32 -> (B,128,NS,PR,PC) fp16 patch layout (edge padded)."""
    Bn = x.shape[0]
    xpad = np.pad(x[:, 0], ((0, 0), (1, 1), (1, 1)), mode="edge").astype(np.float16)
    xprep = np.empty((Bn, 128, NS, PR, PC), np.float16)
    st = xpad.strides
    for q in range(NQ):
        for s in range(NS):
            c0 = 480 * q + SW * s
            block = xpad[:, :, c0:c0 + PC]
            v = np.lib.stride_tricks.as_strided(
                block, shape=(Bn, NB, PR, PC),
                strides=(st[0], BH * st[1], st[1], st[2]))
            xprep[:, q * NB:(q + 1) * NB, s] = v
    return xprep


def _assemble(y, x):
    """y (B,128,NS,8,QR,QC) fp16 planes + x (B,1,H,W) f32 -> (B,3,H,W) f32."""
    Bn = x.shape[0]
    out = np.empty((Bn, 3, H, W), np.float32)
    # identity channels from the exact f32 input
    out[:, 0, 0::2, 0::2] = x[:, 0, 0::2, 0::2]   # R(e,e)
    out[:, 1, 0::2, 1::2] = x[:, 0, 0::2, 1::2]   # G(e,o)
    out[:, 1, 1::2, 0::2] = x[:, 0, 1::2, 0::2]   # G(o,e)
    out[:, 2, 1::2, 1::2] = x[:, 0, 1::2, 1::2]   # B(o,o)
    yv = y.reshape(Bn, NQ, NB, NS, 8, QR, QC)
    for q in range(NQ):
        for s in range(NS):
            c0 = 480 * q + SW * s
            sub = yv[:, q, :, s]                   # (B, NB, 8, QR, QC)
            for pl, ch, rp, cp in PLANES:
                arr = sub[:, :, pl].reshape(Bn, NB * QR, QC)
                out[:, ch, rp::2, c0 + cp:c0 + SW:2] = arr
    return out


def kernel(x, kernels=None, index=None, **_unused):
    global LAST_RESULTS
    x = np.ascontiguousarray(np.asarray(x), dtype=np.float32)
    Bn = x.shape[0]
    xprep = _prep_inputs(x)
    nc = _build()
    from concourse.bass_utils import run_bass_kernel_spmd
    in_maps = [{"xprep": xprep[i]} for i in range(Bn)]
    res = run_bass_kernel_spmd(nc, in_maps, core_ids=list(range(Bn)))
    LAST_RESULTS = res
    y = np.stack([res.results[i]["yout"] for i in range(Bn)])
    return _assemble(y, x)
